# revision 27
# baseline (speedup 1.0000x reference)
"""Bass/Trainium2 kernel for nn_BiLinearDecoder.

Reference computation (per the original nn.Module, inference mode):
    lnc = feature[:U]; dis = feature[U:U+V]; M = feature[U+V:]
    scores[u,v,k] = lnc[u] @ W_k @ dis[v]
    outputs = relu(scores.reshape(U*V, NW) @ weight_classifier)   # [U*V, NC]
    lnc_rep = repeat(lnc, V, axis=0)                              # [U*V, D]
    dis_rep = tile(dis, (U, 1))                                   # [U*V, D]
    returns (outputs, lnc_rep, dis_rep, M)

Strategy (memory-regime problem: ~1.03 GB of output writes dominate):
  - Shard U across the 8 cores (U/8 = 125 rows each); dis + weights replicated.
  - Fold the classifier into the bilinear weights on-device:
        B_c = sum_k wc[k,c] * W_k   ->   outputs[u,v,c] = relu(lnc[u] @ B_c @ dis[v])
    so the per-core compute is 2 matmul chains (one per class).
  - lnc_rep / dis_rep (64 MB each per core) are produced by big SBUF->DRAM DMAs
    from small SBUF tiles that hold the source rows pre-replicated a few times,
    so every DMA moves >=1 MB with >=1 KB contiguous runs.
"""

import numpy as np

SIZE_U = 1000
SIZE_V = 1000
D = 128
NW = 2
NCLS = 2
M_EXTRA = 16
N_CORES = 8
U_SHARD = SIZE_U // N_CORES  # 125


def _build_nc(us=U_SHARD, v=SIZE_V, d=D, nw=NW, ncls=NCLS, m_extra=M_EXTRA):
    """Build the single-core Bass program (SPMD: same program on all cores)."""
    import concourse.bass as bass
    import concourse.mybir as mybir
    from concourse import bacc, masks
    from concourse.tile import TileContext

    f32 = mybir.dt.float32

    VC = min(v, 500)          # matmul free-dim chunk (psum bank = 512 f32)

    # DMA store structure rules (HW-measured):
    #  - per-DMA dest must be fully contiguous AND consecutive descriptors
    #    must cycle source partitions -> all 16 SDMA engines (~240 GB/s)
    #  - exactly-4096B descriptor runs -> 5-engine pathology (~60 GB/s)
    #  - same-partition consecutive descriptors -> SBUF port serialization
    #  - DRAM->DRAM with step-0 (broadcast) source -> 16 engines (~270 GB/s)
    # lnc replicas per partition: largest divisor of v with >=16 broadcast
    # descriptors per D2D store (so each DMA spreads over all 16 engines)
    LNCREP_J = 1
    for _c in range(1, v + 1):
        if v % _c == 0 and _c * d * 4 <= 65536 and v // _c >= 16:
            LNCREP_J = _c

    nc = bacc.Bacc(None, target_bir_lowering=False)

    lnc_in = nc.dram_tensor("lnc_in", [us, d], f32, kind="ExternalInput")
    dis_in = nc.dram_tensor("dis_in", [v, d], f32, kind="ExternalInput")
    w_in = nc.dram_tensor("w_in", [nw, d, d], f32, kind="ExternalInput")
    wc_in = nc.dram_tensor("wc_in", [nw, ncls], f32, kind="ExternalInput")
    m_in = nc.dram_tensor("m_in", [m_extra, d], f32, kind="ExternalInput")

    out_cls = nc.dram_tensor("out_cls", [us * v, ncls], f32, kind="ExternalOutput")
    out_lnc = nc.dram_tensor("out_lnc", [us * v, d], f32, kind="ExternalOutput")
    out_dis = nc.dram_tensor("out_dis", [us * v, d], f32, kind="ExternalOutput")
    out_m = nc.dram_tensor("out_m", [m_extra, d], f32, kind="ExternalOutput")
    # DRAM staging for the lnc_rep broadcast source
    lnc_stage = nc.dram_tensor("lnc_stage", [us * LNCREP_J * d], f32)

    with TileContext(nc) as tc:
        with (
            tc.tile_pool(name="singles", bufs=1) as singles,
            tc.tile_pool(name="chunks", bufs=3) as chunks,
            # Walrus codegen allows only ONE sem-wait on a PE (Matmult)
            # instruction. Every PE input is therefore produced by DVE
            # (single DVE sem), and PSUM pools are sized so no matmul ever
            # waits on an ACT-released slot. Bank budget (8 banks total):
            # ps1(1) + ps2(2) + ps_cct(1) + ps_s(4) = 8.
            tc.tile_pool(name="ps1", bufs=1, space=bass.MemorySpace.PSUM) as ps1,
            tc.tile_pool(name="ps2", bufs=2, space=bass.MemorySpace.PSUM) as ps2,
            tc.tile_pool(name="ps_cct", bufs=1, space=bass.MemorySpace.PSUM) as ps_cct,
            tc.tile_pool(name="ps_s", bufs=4, space=bass.MemorySpace.PSUM) as ps_s,
        ):
            # ---------- loads ----------
            lnc_sb = singles.tile([us, d], f32)
            nc.scalar.dma_start(out=lnc_sb, in_=lnc_in[:])

            # dis_rep: one DRAM->DRAM broadcast DMA straight from dis_in —
            # the 512KB dis block repeated us times into the contiguous
            # 64MB dest (descriptors auto-split to 64KB, all 16 engines)
            dis_src = bass.AP(tensor=dis_in, offset=0,
                              ap=[[0, us], [1, v * d]])
            nc.sync.dma_start(
                out=out_dis.rearrange("(r w) d -> r (w d)", r=us),
                in_=dis_src,
            )

            w_sb = []
            for k in range(nw):
                wt = singles.tile([d, d], f32, tag=f"w{k}")
                nc.scalar.dma_start(out=wt, in_=w_in[k])
                w_sb.append(wt)

            # classifier weights broadcast to all partitions: [128, nw*ncls]
            wc_sb = singles.tile([128, nw * ncls], f32)
            wc_flat = wc_in.rearrange("a b -> (a b)")
            wc_bcast = bass.AP(
                tensor=wc_flat.tensor, offset=wc_flat.offset,
                ap=[[0, 128]] + list(wc_flat.ap),
            )
            nc.gpsimd.dma_start(out=wc_sb, in_=wc_bcast)

            identity_g = singles.tile([128, 128], f32)
            masks.make_identity(nc, identity_g[:])
            # DVE-owned copy so PE transposes wait only on the DVE sem
            identity = singles.tile([128, 128], f32)
            nc.vector.tensor_copy(out=identity, in_=identity_g)

            # M rows pass through (tiny)
            nc.scalar.dma_start(out=out_m[:], in_=m_in[:])

            # ---------- replicate source tiles in SBUF ----------
            # lncrep: LNCREP_J copies of each core-local lnc row, along free dim
            lncrep = singles.tile([us, LNCREP_J * d], f32)
            nc.vector.tensor_copy(out=lncrep[:, :d], in_=lnc_sb)
            filled = 1
            while filled < LNCREP_J:
                n = min(filled, LNCREP_J - filled)
                nc.vector.tensor_copy(
                    out=lncrep[:, filled * d:(filled + n) * d],
                    in_=lncrep[:, : n * d],
                )
                filled += n

            # ---------- compute path: outputs = relu(lnc @ B_c @ dis^T) ----------
            # B_c = wc[0,c]*W_0 + wc[1,c]*W_1  (general nw via accumulation)
            b_sb = []
            for c in range(ncls):
                acc = singles.tile([d, d], f32, tag=f"b{c}")
                tmp = chunks.tile([d, d], f32, tag="btmp")
                nc.vector.tensor_scalar_mul(acc, w_sb[0], wc_sb[:d, c:c + 1])
                for k in range(1, nw):
                    idx = k * ncls + c
                    nc.vector.tensor_scalar_mul(tmp, w_sb[k], wc_sb[:d, idx:idx + 1])
                    nc.vector.tensor_add(out=acc, in0=acc, in1=tmp)
                b_sb.append(acc)

            # lncT [d, us] via PE transpose (input = DVE-written lncrep slot 0)
            lncT_ps = ps1.tile([d, us], f32, tag="lncT_ps")
            nc.tensor.transpose(lncT_ps, lncrep[:, :d], identity[:us, :us])
            lncT = singles.tile([d, us], f32)
            nc.vector.tensor_copy(out=lncT, in_=lncT_ps)

            # disT [d, v] via PE transposes of 128-row chunks
            disT = singles.tile([d, v], f32)
            nchunk = (v + 127) // 128
            for j in range(nchunk):
                v0 = j * 128
                p = min(128, v - v0)
                dchunk = chunks.tile([128, d], f32, tag="dchunk")
                nc.scalar.dma_start(out=dchunk[:p], in_=dis_in[v0:v0 + p])
                dchunk2 = chunks.tile([128, d], f32, tag="dchunk2")
                nc.vector.tensor_copy(out=dchunk2[:p], in_=dchunk[:p])
                t_ps = ps2.tile([d, 128], f32, tag="t_ps")
                nc.tensor.transpose(t_ps[:, :p], dchunk2[:p], identity[:p, :p])
                nc.vector.tensor_copy(out=disT[:, v0:v0 + p], in_=t_ps[:, :p])

            # out_sb [us, v*ncls], interleaved (v, c) layout matching DRAM
            out_sb = singles.tile([us, v * ncls], f32)
            out_sb3 = out_sb.rearrange("p (w c) -> p w c", c=ncls)

            for c in range(ncls):
                # C_cT [d(e), us] = B_c^T-contract: sum_d B_c[d,e] lncT[d,u]
                cct_ps = ps_cct.tile([d, us], f32, tag="cct_ps")
                nc.tensor.matmul(cct_ps, b_sb[c], lncT)
                cct = singles.tile([d, us], f32, tag=f"cct{c}")
                nc.vector.tensor_copy(out=cct, in_=cct_ps)

                for v0 in range(0, v, VC):
                    w_ = min(VC, v - v0)
                    s_ps = ps_s.tile([us, VC], f32, tag="s_ps")
                    nc.tensor.matmul(s_ps[:, :w_], cct, disT[:, v0:v0 + w_])
                    # relu PSUM -> strided SBUF slot (free-dim stride = ncls)
                    nc.scalar.activation(
                        out=out_sb3[:, v0:v0 + w_, c],
                        in_=s_ps[:, :w_],
                        func=mybir.ActivationFunctionType.Relu,
                    )

            # out_cls on the gpsimd (SWDGE) queue: it depends on the whole
            # compute chain and must not head-of-line-block the big
            # HWDGE store streams
            nc.gpsimd.dma_start(
                out=out_cls.rearrange("(u w) c -> u (w c)", u=us),
                in_=out_sb,
            )

            # ---------- big replicated stores ----------
            # Fully-contiguous DRAM destinations engage all 16 SDMA engines
            # (~360 GB/s); 2D-strided dests fall into a 5-engine ring set
            # (~130 GB/s). Both stores below merge to flat dest APs.

            # lnc_rep in two hops (on the scalar HWDGE queue so it streams
            # concurrently with the dis broadcast on the sync queue):
            #  1) stage the J-replica row groups to DRAM (contiguous store,
            #     partition-cycling descriptors)
            #  2) per u, one DRAM->DRAM broadcast DMA: the staged row group
            #     repeated v/J (>=16) times into u's contiguous dest block
            ngrp = v // LNCREP_J
            stage_view = lnc_stage.rearrange("(u f) -> u f", u=us)
            nc.scalar.dma_start(out=stage_view, in_=lncrep)
            lnc_view = out_lnc.rearrange("(u w) d -> u (w d)", u=us)
            for u in range(us):
                src_u = bass.AP(
                    tensor=lnc_stage, offset=u * LNCREP_J * d,
                    ap=[[0, ngrp], [1, LNCREP_J * d]],
                )
                nc.scalar.dma_start(out=lnc_view[u], in_=src_u)

    nc.compile()  # bacc passes: legalizes the 1-sem-wait-per-instruction limit
    return nc


_NC_CACHE = {}

# Dev/profiling hooks (unused by the grading path): set TRACE=True before
# calling kernel() to capture an NTFF profile; results land in LAST_RESULT.
TRACE = False
LAST_RESULT = None


def _get_nc(key, **kw):
    if key not in _NC_CACHE:
        _NC_CACHE[key] = _build_nc(**kw)
    return _NC_CACHE[key]


def kernel(feature, weight, weight_classifier):
    from concourse.bass_utils import run_bass_kernel_spmd

    feature = np.ascontiguousarray(feature, dtype=np.float32)
    weight = np.ascontiguousarray(weight, dtype=np.float32)
    weight_classifier = np.ascontiguousarray(weight_classifier, dtype=np.float32)

    lnc = feature[:SIZE_U]
    dis = feature[SIZE_U:SIZE_U + SIZE_V]
    m = feature[SIZE_U + SIZE_V:]

    nc = _get_nc("full")

    in_maps = []
    for i in range(N_CORES):
        in_maps.append({
            "lnc_in": np.ascontiguousarray(lnc[i * U_SHARD:(i + 1) * U_SHARD]),
            "dis_in": dis,
            "w_in": weight,
            "wc_in": weight_classifier,
            "m_in": m,
        })

    res = run_bass_kernel_spmd(
        nc, in_maps, core_ids=list(range(N_CORES)), trace=TRACE
    )
    global LAST_RESULT
    LAST_RESULT = res
    outs = res.results

    outputs = np.concatenate([outs[i]["out_cls"] for i in range(N_CORES)], axis=0)
    lnc_rep = np.concatenate([outs[i]["out_lnc"] for i in range(N_CORES)], axis=0)
    dis_rep = np.concatenate([outs[i]["out_dis"] for i in range(N_CORES)], axis=0)
    m_out = outs[0]["out_m"]

    return (outputs, lnc_rep, dis_rep, m_out)


# revision 29
# speedup vs baseline: 1.2936x; 1.2936x over previous
"""Bass/Trainium2 kernel for nn_BiLinearDecoder.

Reference computation (per the original nn.Module, inference mode):
    lnc = feature[:U]; dis = feature[U:U+V]; M = feature[U+V:]
    scores[u,v,k] = lnc[u] @ W_k @ dis[v]
    outputs = relu(scores.reshape(U*V, NW) @ weight_classifier)   # [U*V, NC]
    lnc_rep = repeat(lnc, V, axis=0)                              # [U*V, D]
    dis_rep = tile(dis, (U, 1))                                   # [U*V, D]
    returns (outputs, lnc_rep, dis_rep, M)

Strategy (memory-regime problem: ~1.03 GB of output writes dominate):
  - Shard U across the 8 cores (U/8 = 125 rows each); dis + weights replicated.
  - Fold the classifier into the bilinear weights on-device:
        B_c = sum_k wc[k,c] * W_k   ->   outputs[u,v,c] = relu(lnc[u] @ B_c @ dis[v])
    so the per-core compute is 2 matmul chains (one per class).
  - lnc_rep / dis_rep (64 MB each per core) are produced by big SBUF->DRAM DMAs
    from small SBUF tiles that hold the source rows pre-replicated a few times,
    so every DMA moves >=1 MB with >=1 KB contiguous runs.
"""

import numpy as np

SIZE_U = 1000
SIZE_V = 1000
D = 128
NW = 2
NCLS = 2
M_EXTRA = 16
N_CORES = 8
U_SHARD = SIZE_U // N_CORES  # 125


def _build_nc(us=U_SHARD, v=SIZE_V, d=D, nw=NW, ncls=NCLS, m_extra=M_EXTRA):
    """Build the single-core Bass program (SPMD: same program on all cores)."""
    import concourse.bass as bass
    import concourse.mybir as mybir
    from concourse import bacc, masks
    from concourse.tile import TileContext

    f32 = mybir.dt.float32

    VC = min(v, 500)          # matmul free-dim chunk (psum bank = 512 f32)

    # DMA store structure rules (HW-measured):
    #  - per-DMA dest must be fully contiguous AND consecutive descriptors
    #    must cycle source partitions -> all 16 SDMA engines (~240 GB/s)
    #  - exactly-4096B descriptor runs -> 5-engine pathology (~60 GB/s)
    #  - same-partition consecutive descriptors -> SBUF port serialization
    #  - DRAM->DRAM with step-0 (broadcast) source -> 16 engines (~270 GB/s)
    # lnc replicas per partition: largest divisor of v with >=16 broadcast
    # descriptors per D2D store (so each DMA spreads over all 16 engines)
    LNCREP_J = 1
    for _c in range(1, v + 1):
        if v % _c == 0 and _c * d * 4 <= 65536 and v // _c >= 16:
            LNCREP_J = _c

    nc = bacc.Bacc(None, target_bir_lowering=False)

    lnc_in = nc.dram_tensor("lnc_in", [us, d], f32, kind="ExternalInput")
    dis_in = nc.dram_tensor("dis_in", [v, d], f32, kind="ExternalInput")
    w_in = nc.dram_tensor("w_in", [nw, d, d], f32, kind="ExternalInput")
    wc_in = nc.dram_tensor("wc_in", [nw, ncls], f32, kind="ExternalInput")
    m_in = nc.dram_tensor("m_in", [m_extra, d], f32, kind="ExternalInput")

    out_cls = nc.dram_tensor("out_cls", [us * v, ncls], f32, kind="ExternalOutput")
    out_lnc = nc.dram_tensor("out_lnc", [us * v, d], f32, kind="ExternalOutput")
    out_dis = nc.dram_tensor("out_dis", [us * v, d], f32, kind="ExternalOutput")
    out_m = nc.dram_tensor("out_m", [m_extra, d], f32, kind="ExternalOutput")
    # DRAM staging for the lnc_rep broadcast source
    lnc_stage = nc.dram_tensor("lnc_stage", [us * LNCREP_J * d], f32)

    with TileContext(nc) as tc:
        with (
            tc.tile_pool(name="singles", bufs=1) as singles,
            tc.tile_pool(name="chunks", bufs=3) as chunks,
            # Walrus codegen allows only ONE sem-wait on a PE (Matmult)
            # instruction. Every PE input is therefore produced by DVE
            # (single DVE sem), and PSUM pools are sized so no matmul ever
            # waits on an ACT-released slot. Bank budget (8 banks total):
            # ps1(1) + ps2(2) + ps_cct(1) + ps_s(4) = 8.
            tc.tile_pool(name="ps1", bufs=1, space=bass.MemorySpace.PSUM) as ps1,
            tc.tile_pool(name="ps2", bufs=2, space=bass.MemorySpace.PSUM) as ps2,
            tc.tile_pool(name="ps_cct", bufs=1, space=bass.MemorySpace.PSUM) as ps_cct,
            tc.tile_pool(name="ps_s", bufs=4, space=bass.MemorySpace.PSUM) as ps_s,
        ):
            # ---------- loads ----------
            lnc_sb = singles.tile([us, d], f32)
            nc.scalar.dma_start(out=lnc_sb, in_=lnc_in[:])

            # dis block staged in SBUF as [64, v*d/64]: flat byte order split
            # across 64 partitions (8000B runs for v=1000)
            DP = 64
            assert (v * d) % DP == 0
            dis_sb = singles.tile([DP, v * d // DP], f32)
            nc.sync.dma_start(
                out=dis_sb,
                in_=dis_in.rearrange("v d -> (v d)").rearrange(
                    "(p f) -> p f", p=DP),
            )

            w_sb = []
            for k in range(nw):
                wt = singles.tile([d, d], f32, tag=f"w{k}")
                nc.scalar.dma_start(out=wt, in_=w_in[k])
                w_sb.append(wt)

            # classifier weights broadcast to all partitions: [128, nw*ncls]
            wc_sb = singles.tile([128, nw * ncls], f32)
            wc_flat = wc_in.rearrange("a b -> (a b)")
            wc_bcast = bass.AP(
                tensor=wc_flat.tensor, offset=wc_flat.offset,
                ap=[[0, 128]] + list(wc_flat.ap),
            )
            nc.gpsimd.dma_start(out=wc_sb, in_=wc_bcast)

            identity_g = singles.tile([128, 128], f32)
            masks.make_identity(nc, identity_g[:])
            # DVE-owned copy so PE transposes wait only on the DVE sem
            identity = singles.tile([128, 128], f32)
            nc.vector.tensor_copy(out=identity, in_=identity_g)

            # M rows pass through (tiny)
            nc.scalar.dma_start(out=out_m[:], in_=m_in[:])

            # ---------- replicate source tiles in SBUF ----------
            # lncrep: LNCREP_J copies of each core-local lnc row, along free dim
            lncrep = singles.tile([us, LNCREP_J * d], f32)
            nc.vector.tensor_copy(out=lncrep[:, :d], in_=lnc_sb)
            filled = 1
            while filled < LNCREP_J:
                n = min(filled, LNCREP_J - filled)
                nc.vector.tensor_copy(
                    out=lncrep[:, filled * d:(filled + n) * d],
                    in_=lncrep[:, : n * d],
                )
                filled += n

            # ---------- compute path: outputs = relu(lnc @ B_c @ dis^T) ----------
            # B_c = wc[0,c]*W_0 + wc[1,c]*W_1  (general nw via accumulation)
            b_sb = []
            for c in range(ncls):
                acc = singles.tile([d, d], f32, tag=f"b{c}")
                tmp = chunks.tile([d, d], f32, tag="btmp")
                nc.vector.tensor_scalar_mul(acc, w_sb[0], wc_sb[:d, c:c + 1])
                for k in range(1, nw):
                    idx = k * ncls + c
                    nc.vector.tensor_scalar_mul(tmp, w_sb[k], wc_sb[:d, idx:idx + 1])
                    nc.vector.tensor_add(out=acc, in0=acc, in1=tmp)
                b_sb.append(acc)

            # lncT [d, us] via PE transpose (input = DVE-written lncrep slot 0)
            lncT_ps = ps1.tile([d, us], f32, tag="lncT_ps")
            nc.tensor.transpose(lncT_ps, lncrep[:, :d], identity[:us, :us])
            lncT = singles.tile([d, us], f32)
            nc.vector.tensor_copy(out=lncT, in_=lncT_ps)

            # disT [d, v] via PE transposes of 128-row chunks
            disT = singles.tile([d, v], f32)
            nchunk = (v + 127) // 128
            for j in range(nchunk):
                v0 = j * 128
                p = min(128, v - v0)
                dchunk = chunks.tile([128, d], f32, tag="dchunk")
                nc.scalar.dma_start(out=dchunk[:p], in_=dis_in[v0:v0 + p])
                dchunk2 = chunks.tile([128, d], f32, tag="dchunk2")
                nc.vector.tensor_copy(out=dchunk2[:p], in_=dchunk[:p])
                t_ps = ps2.tile([d, 128], f32, tag="t_ps")
                nc.tensor.transpose(t_ps[:, :p], dchunk2[:p], identity[:p, :p])
                nc.vector.tensor_copy(out=disT[:, v0:v0 + p], in_=t_ps[:, :p])

            # out_sb [us, v*ncls], interleaved (v, c) layout matching DRAM
            out_sb = singles.tile([us, v * ncls], f32)
            out_sb3 = out_sb.rearrange("p (w c) -> p w c", c=ncls)

            for c in range(ncls):
                # C_cT [d(e), us] = B_c^T-contract: sum_d B_c[d,e] lncT[d,u]
                cct_ps = ps_cct.tile([d, us], f32, tag="cct_ps")
                nc.tensor.matmul(cct_ps, b_sb[c], lncT)
                cct = singles.tile([d, us], f32, tag=f"cct{c}")
                nc.vector.tensor_copy(out=cct, in_=cct_ps)

                for v0 in range(0, v, VC):
                    w_ = min(VC, v - v0)
                    s_ps = ps_s.tile([us, VC], f32, tag="s_ps")
                    nc.tensor.matmul(s_ps[:, :w_], cct, disT[:, v0:v0 + w_])
                    # relu PSUM -> strided SBUF slot (free-dim stride = ncls)
                    nc.scalar.activation(
                        out=out_sb3[:, v0:v0 + w_, c],
                        in_=s_ps[:, :w_],
                        func=mybir.ActivationFunctionType.Relu,
                    )

            # out_cls on the gpsimd (SWDGE) queue: it depends on the whole
            # compute chain and must not head-of-line-block the big
            # HWDGE store streams
            nc.gpsimd.dma_start(
                out=out_cls.rearrange("(u w) c -> u (w c)", u=us),
                in_=out_sb,
            )

            # ---------- big replicated stores ----------
            # Fully-contiguous DRAM destinations engage all 16 SDMA engines
            # (~360 GB/s); 2D-strided dests fall into a 5-engine ring set
            # (~130 GB/s). Both stores below merge to flat dest APs.

            # dis_rep: one 512KB contiguous store per replica (16-engine
            # spread: contiguous dest, partition-cycling 8000B descriptors)
            dis_view = out_dis.rearrange("(r w) d -> r (w d)", r=us)
            for r in range(us):
                nc.sync.dma_start(out=dis_view[r], in_=dis_sb)

            # lnc_rep in two hops:
            #  1) stage the J-replica row groups to DRAM (contiguous store,
            #     partition-cycling descriptors)
            #  2) per u, one DRAM->DRAM broadcast DMA: the staged row group
            #     repeated v/J (>=16 descriptors) times into u's contiguous
            #     dest block
            ngrp = v // LNCREP_J
            stage_view = lnc_stage.rearrange("(u f) -> u f", u=us)
            nc.sync.dma_start(out=stage_view, in_=lncrep)
            lnc_view = out_lnc.rearrange("(u w) d -> u (w d)", u=us)
            for u in range(us):
                src_u = bass.AP(
                    tensor=lnc_stage, offset=u * LNCREP_J * d,
                    ap=[[0, ngrp], [1, LNCREP_J * d]],
                )
                nc.sync.dma_start(out=lnc_view[u], in_=src_u)

    nc.compile()  # bacc passes: legalizes the 1-sem-wait-per-instruction limit
    return nc


_NC_CACHE = {}

# Dev/profiling hooks (unused by the grading path): set TRACE=True before
# calling kernel() to capture an NTFF profile; results land in LAST_RESULT.
TRACE = False
LAST_RESULT = None


def _get_nc(key, **kw):
    if key not in _NC_CACHE:
        _NC_CACHE[key] = _build_nc(**kw)
    return _NC_CACHE[key]


def kernel(feature, weight, weight_classifier):
    from concourse.bass_utils import run_bass_kernel_spmd

    feature = np.ascontiguousarray(feature, dtype=np.float32)
    weight = np.ascontiguousarray(weight, dtype=np.float32)
    weight_classifier = np.ascontiguousarray(weight_classifier, dtype=np.float32)

    lnc = feature[:SIZE_U]
    dis = feature[SIZE_U:SIZE_U + SIZE_V]
    m = feature[SIZE_U + SIZE_V:]

    nc = _get_nc("full")

    in_maps = []
    for i in range(N_CORES):
        in_maps.append({
            "lnc_in": np.ascontiguousarray(lnc[i * U_SHARD:(i + 1) * U_SHARD]),
            "dis_in": dis,
            "w_in": weight,
            "wc_in": weight_classifier,
            "m_in": m,
        })

    res = run_bass_kernel_spmd(
        nc, in_maps, core_ids=list(range(N_CORES)), trace=TRACE
    )
    global LAST_RESULT
    LAST_RESULT = res
    outs = res.results

    outputs = np.concatenate([outs[i]["out_cls"] for i in range(N_CORES)], axis=0)
    lnc_rep = np.concatenate([outs[i]["out_lnc"] for i in range(N_CORES)], axis=0)
    dis_rep = np.concatenate([outs[i]["out_dis"] for i in range(N_CORES)], axis=0)
    m_out = outs[0]["out_m"]

    return (outputs, lnc_rep, dis_rep, m_out)
